# revision 47
# baseline (speedup 1.0000x reference)
"""Trainium2 Bass kernel for nn_Block_41077067219413.

Reference computation (B=2048, D=dim_in=4096, J=dim_out=4096):
    xf = x.astype(f32)                 # (B, D) in {0,1}
    mf = masks.astype(f32)             # (D, J) in {0,1}
    sums = xf @ mf + (1-xf) @ (1-mf)   # XNOR popcount over D
    out  = sums > thresholds[None, :]  # (B, J) bool

Identity: with x' = 2x-1 in {-1,+1}, m in {0,1}, A = x' @ m:
    sums = A + D - rowsum_x   (colsum terms cancel)
    out  = A > th[j] + rowsum_x[b] - D

Sharding: 4 batch groups x 2 j-halves across 8 cores.  Per core one fp8
DoubleRow GEMM [512 x 4096] @ [4096 x 2048] -- 256 matmuls of
[K=256]x[N=512], the PE-array floor (~55us at 157 TF/s fp8-DR).
Everything else is kept off the PE:
  - x is host-marshalled to the exact stationary tile layout (fp8 +-1,
    transposed, DR k-pairing) -- no on-device transposes/converts.
  - masks DMA'd raw as uint8 in k-pair tile layout, bitcast to fp8
    (byte 0x01 == eps = 2^-9 subnormal); psum accumulates eps*A exactly.
  - thresholds ship as an eps-scaled f32 broadcast tile; epilogue is
    tmp = psum - eps*th (DVE, releases the psum bank) then
    out = tmp > eps*(rowsum-D), all integer-exact.
  - rowsum_x from a row-major fp8 x copy via accum-reductions on the
    scalar/vector engines mid-flight.
PSUM: two waves of 8 banks split by LOCAL J-HALF (4 b-tiles x 2
j-tiles each) so wave A only needs half the mask bytes plus all xT
(~6.5 MB) inside its span -- 2.7x DMA-bandwidth slack against HBM
jitter.  kp 0..11 kp-major, kp 12..15 group-major so groups retire
staggered.  Dummy warm-up matmuls ramp the PE p-state while the first
tiles land.
"""

import numpy as np

B, D, J = 2048, 4096, 4096
NCORES = 8
GB = 4                    # batch groups
GJ = 2                    # j halves (across cores)
ML = B // GB              # 512 rows per core
JL = J // GJ              # 2048 cols per core
JW = JL // 2              # 1024 cols per wave (local j-half)
P = 128
NB = ML // P              # 4 b-tiles per core
KP = D // 256             # 16 k-pair tiles
JN = 512                  # one PSUM bank
KRET = 4                  # retirement kps (12..15)
WARM = 28                 # PE p-state warm-up matmuls
# mask DMA chunks (in kp units): small first chunks for a fast start,
# 2-kp chunks after to keep the issue count low
MCHUNKS = (1, 1, 2, 2, 2, 2, 2, 2, 2)
# xT DMA chunks (in kp units): small first chunk so the first matmul's
# stationary lands with the first mask chunk
XCHUNKS = (2, 2, 4, 4, 4)

_cache = {}


def _build():
    import concourse.bacc as bacc
    import concourse.mybir as mybir
    import concourse.tile as tile

    dt = mybir.dt
    f8 = dt.float8e4
    f32 = dt.float32
    AF = mybir.ActivationFunctionType
    ALU = mybir.AluOpType
    DR = mybir.MatmulPerfMode.DoubleRow

    nc = bacc.Bacc("TRN2", target_bir_lowering=False, debug=False,
                   num_devices=NCORES)

    xT_ds = [nc.dram_tensor(f"xt{c}", [P, NB, nk, 2, P], f8,
                            kind="ExternalInput")
             for c, nk in enumerate(XCHUNKS)]
    xrm_d = nc.dram_tensor("xrm", [NB, P, D], f8, kind="ExternalInput")
    m_d = nc.dram_tensor("masks", [2, KP, P, 2, JW], dt.uint8,
                         kind="ExternalInput")
    thb_d = nc.dram_tensor("thb", [P, JL], f32, kind="ExternalInput")
    o_d = nc.dram_tensor("out", [NB, P, JL], dt.uint8, kind="ExternalOutput")

    with tile.TileContext(nc) as tc:
        with (
            tc.tile_pool(name="const", bufs=1) as constp,
            tc.tile_pool(name="mask", bufs=1) as maskp,
            tc.tile_pool(name="xt", bufs=1) as xtp,
            tc.tile_pool(name="xrm", bufs=1) as xrmp,
            tc.tile_pool(name="acts", bufs=1) as actp,
            tc.tile_pool(name="tmp", bufs=1) as tmpp,
            tc.tile_pool(name="bound", bufs=1) as boundp,
            tc.tile_pool(name="ob", bufs=1) as obsp,
        ):
            mtc = [[maskp.tile([P, nk, 2, JW], dt.uint8,
                               name=f"mk{hh}_{c}")
                    for c, nk in enumerate(MCHUNKS)] for hh in range(2)]
            kp2c = []
            for c, nk in enumerate(MCHUNKS):
                kp2c += [(c, i) for i in range(nk)]
            xq = [xtp.tile([P, NB, nk, 2, P], f8, name=f"xq{q}")
                  for q, nk in enumerate(XCHUNKS)]
            xbase = []
            k0 = 0
            for nk in XCHUNKS:
                xbase.append(k0)
                k0 += nk
            xrm01 = xrmp.tile([P, 2, D], f8)
            xrm23 = xrmp.tile([P, 2, D], f8)
            thb = constp.tile([P, JL], f32)

            def mte(hh, kp, jj):
                c, i = kp2c[kp]
                return mtc[hh][c][:, i, :, jj:jj + JN]

            def xte(b, kp):
                for q in range(len(XCHUNKS) - 1, -1, -1):
                    if kp >= xbase[q]:
                        return xq[q][:, b, kp - xbase[q]]

            # DMA order: first xT chunk on the scalar queue (lands in
            # parallel with the first mask chunk); the strictly ordered
            # gpsimd queue carries wave-A (j-half 0) masks with the
            # remaining xT chunks slotted before the kp range needing
            # them, then thb/xrm, then wave-B masks.
            nc.scalar.dma_start(xq[0][:], xT_ds[0][:])
            for c, nk in enumerate(MCHUNKS):
                kp0 = kp2c.index((c, 0))
                for q, xb in enumerate(xbase):
                    if q > 0 and xb == kp0:
                        nc.gpsimd.dma_start(xq[q][:], xT_ds[q][:])
                nc.gpsimd.dma_start(
                    mtc[0][c][:], m_d[0, kp0:kp0 + nk].rearrange(
                        "k p a j -> p k a j"))
            nc.gpsimd.dma_start(thb[:], thb_d[:])
            nc.gpsimd.dma_start(
                xrm01[:], xrm_d[0:2].rearrange("b p k -> p b k"))
            nc.gpsimd.dma_start(
                xrm23[:], xrm_d[2:4].rearrange("b p k -> p b k"))
            for c, nk in enumerate(MCHUNKS):
                kp0 = kp2c.index((c, 0))
                nc.gpsimd.dma_start(
                    mtc[1][c][:], m_d[1, kp0:kp0 + nk].rearrange(
                        "k p a j -> p k a j"))

            # ---- constants / warm-up
            wtile = constp.tile([P, 2, P], f8)
            nc.vector.memset(wtile[:], 0.0)
            zero1 = constp.tile([P, 1], f32)
            nc.vector.memset(zero1[:], 0.0)
            neg4 = constp.tile([P, 1], f32)
            nc.vector.memset(neg4[:], -4.0)
            actw = constp.tile([P, 1], f32)
            nc.scalar.activation(actw[:], zero1[:], AF.Identity,
                                 bias=zero1[:], scale=1.0)

            rxa = [constp.tile([P, 1], f32, name=f"rxa{b}")
                   for b in range(NB)]
            rxe = [constp.tile([P, 1], f32, name=f"rxe{b}")
                   for b in range(NB)]
            sc8 = [actp.tile([P, D], f8, name=f"sc8_{i}") for i in range(3)]

            # rowsum reductions: b0/b1 serial on scalar, b2/b3 on vector
            # (emitted later, after the wave-A psum-releasing ops).
            for b in (0, 1):
                nc.scalar.activation(sc8[b][:], xrm01[:, b], AF.Identity,
                                     bias=zero1[:], scale=1.0,
                                     accum_out=rxa[b][:])
                nc.scalar.activation(rxe[b][:], rxa[b][:], AF.Identity,
                                     bias=neg4[:], scale=1.0 / 1024.0)

            obs = [obsp.tile([P, JL], dt.uint8, name=f"ob{b}")
                   for b in range(NB)]

            with tc.tile_pool(name="psacc", bufs=1, space="PSUM") as psacc:
                dps = psacc.tile([P, JN], f32, tag="acc0", name="dps")
                for i in range(WARM):
                    nc.tensor.matmul(dps[:, 0:P], wtile[:], wtile[:],
                                     start=True, stop=True, perf_mode=DR)

                for w in range(2):
                    hh = w
                    ps = {}
                    for b in range(NB):
                        for jl in range(2):
                            ps[(b, jl)] = psacc.tile(
                                [P, JN], f32, tag=f"acc{b * 2 + jl}",
                                name=f"acc_w{w}_{b}_{jl}")
                    # bulk: kp-major over kp 0..11
                    for kp in range(KP - KRET):
                        for b in range(NB):
                            wap = xte(b, kp)
                            for jl in range(2):
                                nc.tensor.matmul(
                                    ps[(b, jl)][:], wap,
                                    mte(hh, kp, jl * JN).bitcast(f8),
                                    start=(kp == 0), stop=False,
                                    perf_mode=DR)
                    if w == 1:
                        # pre-run the final group's kp 12..14 so the very
                        # last retirement is a single matmul -- shortens
                        # the end-of-kernel tail
                        for kp in range(KP - KRET, KP - 1):
                            nc.tensor.matmul(
                                ps[(NB - 1, 1)][:], xte(NB - 1, kp),
                                mte(hh, kp, JN).bitcast(f8),
                                start=False, stop=False, perf_mode=DR)
                    # retirement: group-major over kp 12..15, staggered
                    tmps = []
                    for b in range(NB):
                        for jl in range(2):
                            col = hh * JW + jl * JN
                            last = (w == 1 and b == NB - 1 and jl == 1)
                            kp_lo = KP - 1 if last else KP - KRET
                            for kp in range(kp_lo, KP):
                                nc.tensor.matmul(
                                    ps[(b, jl)][:], xte(b, kp),
                                    mte(hh, kp, jl * JN).bitcast(f8),
                                    start=False, stop=(kp == KP - 1),
                                    perf_mode=DR)
                            if w == 0:
                                # two-op epilogue: op1 (DVE) releases the
                                # psum bank using only thb; op2 (deferred
                                # below) waits on the rowsum path.
                                tmp = tmpp.tile([P, JN], f32,
                                                tag=f"tmp{b * 2 + jl}",
                                                name=f"tmp{b}_{jl}")
                                nc.vector.tensor_tensor(
                                    tmp[:], ps[(b, jl)][:],
                                    thb[:, col:col + JN], op=ALU.subtract)
                                tmps.append((b, col, tmp))
                            else:
                                nc.vector.tensor_tensor(
                                    obs[b][:, col:col + JN], ps[(b, jl)][:],
                                    bound[(b, jl)][:], op=ALU.is_gt)
                                nc.sync.dma_start(o_d[b, :, col:col + JN],
                                                  obs[b][:, col:col + JN])
                    if w == 0:
                        # b2/b3 rowsums, wave-A op2s, wave-B bound tiles
                        # -- all on DVE, emitted after wave-A's op1s so
                        # they can't head-of-line block the psum-bank
                        # releases.
                        for i, b in enumerate((2, 3)):
                            nc.vector.tensor_scalar(
                                sc8[2][:], xrm23[:, i], 1.0, 0.0,
                                op0=ALU.mult, op1=ALU.add,
                                accum_out=rxa[b][:])
                            nc.vector.tensor_scalar(
                                rxe[b][:], rxa[b][:], 1.0 / 1024.0, -4.0,
                                op0=ALU.mult, op1=ALU.add)
                        for b, col, tmp in tmps:
                            nc.vector.tensor_scalar(
                                obs[b][:, col:col + JN], tmp[:],
                                rxe[b][:], None, op0=ALU.is_gt)
                            nc.sync.dma_start(o_d[b, :, col:col + JN],
                                              obs[b][:, col:col + JN])
                        bound = {}
                        for b in range(NB):
                            for jl in range(2):
                                col = JW + jl * JN
                                bt = boundp.tile([P, JN], f32,
                                                 name=f"bnd{b}_{jl}")
                                nc.vector.tensor_scalar(
                                    bt[:], thb[:, col:col + JN],
                                    rxe[b][:], None, op0=ALU.add)
                                bound[(b, jl)] = bt

    nc.compile()
    return nc


def _get_nc():
    if "nc" not in _cache:
        _cache["nc"] = _build()
    return _cache["nc"]


def _prep_core(xs8, mask_buf, thb_buf):
    """Per-core input dict from the fp8 x slice and shared mask/th bufs."""
    out = {
        "xrm": np.ascontiguousarray(xs8.reshape(NB, P, D)),
        "masks": mask_buf,
        "thb": thb_buf,
    }
    xb = 0
    for c, nk in enumerate(XCHUNKS):
        sl = xs8[:, xb * 256:(xb + nk) * 256].reshape(NB, P, nk, 2, P)
        out[f"xt{c}"] = np.ascontiguousarray(
            sl.transpose(4, 0, 2, 3, 1))         # [ki, b, kc, ko, m]
        xb += nk
    return out


def run(x, masks, thresholds, trace=False):
    """Run the SPMD kernel on 8 cores. Returns (out_bool, results)."""
    import ml_dtypes
    from concourse.bass_utils import run_bass_kernel_spmd

    nc = _get_nc()
    f8 = ml_dtypes.float8_e4m3

    xs8_all = np.where(np.asarray(x) != 0, np.float32(1.0),
                       np.float32(-1.0)).astype(f8)
    m_u8 = np.ascontiguousarray(np.asarray(masks).view(np.uint8))
    th = np.asarray(thresholds).astype(np.float32) * np.float32(2.0 ** -9)

    mask_bufs, thb_bufs = [], []
    for h in range(GJ):
        mh = m_u8[:, h * JL:(h + 1) * JL].reshape(KP, 2, P, 2, JW)
        mask_bufs.append(np.ascontiguousarray(
            mh.transpose(3, 0, 2, 1, 4)))        # [hh, kp, ki, ko, j]
        thb_bufs.append(np.ascontiguousarray(
            np.broadcast_to(th[None, h * JL:(h + 1) * JL], (P, JL))))

    in_maps = []
    for c in range(NCORES):
        g, h = c // GJ, c % GJ
        in_maps.append(_prep_core(xs8_all[g * ML:(g + 1) * ML],
                                  mask_bufs[h], thb_bufs[h]))

    res = run_bass_kernel_spmd(nc, in_maps, core_ids=list(range(NCORES)),
                               trace=trace)
    out = np.empty((B, J), dtype=np.uint8)
    for c in range(NCORES):
        g, h = c // GJ, c % GJ
        out[g * ML:(g + 1) * ML, h * JL:(h + 1) * JL] = \
            res.results[c]["out"].reshape(ML, JL)
    return out.view(np.bool_), res


def kernel(x, masks, thresholds):
    x = np.asarray(x)
    masks = np.asarray(masks)
    thresholds = np.asarray(thresholds)
    out, _ = run(x, masks, thresholds, trace=False)
    return out


# revision 48
# speedup vs baseline: 1.0112x; 1.0112x over previous
"""Trainium2 Bass kernel for nn_Block_41077067219413.

Reference computation (B=2048, D=dim_in=4096, J=dim_out=4096):
    xf = x.astype(f32)                 # (B, D) in {0,1}
    mf = masks.astype(f32)             # (D, J) in {0,1}
    sums = xf @ mf + (1-xf) @ (1-mf)   # XNOR popcount over D
    out  = sums > thresholds[None, :]  # (B, J) bool

Identity: with x' = 2x-1 in {-1,+1}, m in {0,1}, A = x' @ m:
    sums = A + D - rowsum_x   (colsum terms cancel)
    out  = A > th[j] + rowsum_x[b] - D

Sharding: 4 batch groups x 2 j-halves across 8 cores.  Per core one fp8
DoubleRow GEMM [512 x 4096] @ [4096 x 2048] -- 256 matmuls of
[K=256]x[N=512], the PE-array floor (~55us at 157 TF/s fp8-DR).
Everything else is kept off the PE:
  - x is host-marshalled to the exact stationary tile layout (fp8 +-1,
    transposed, DR k-pairing) -- no on-device transposes/converts.
  - masks DMA'd raw as uint8 in k-pair tile layout, bitcast to fp8
    (byte 0x01 == eps = 2^-9 subnormal); psum accumulates eps*A exactly.
  - thresholds ship as an eps-scaled f32 broadcast tile; epilogue is
    tmp = psum - eps*th (DVE, releases the psum bank) then
    out = tmp > eps*(rowsum-D), all integer-exact.
  - rowsum_x from a row-major fp8 x copy via accum-reductions on the
    scalar/vector engines mid-flight.
PSUM: two waves of 8 banks split by LOCAL J-HALF (4 b-tiles x 2
j-tiles each) so wave A only needs half the mask bytes plus all xT
(~6.5 MB) inside its span -- 2.7x DMA-bandwidth slack against HBM
jitter.  kp 0..11 kp-major, kp 12..15 group-major so groups retire
staggered.  Dummy warm-up matmuls ramp the PE p-state while the first
tiles land.
"""

import numpy as np

B, D, J = 2048, 4096, 4096
NCORES = 8
GB = 4                    # batch groups
GJ = 2                    # j halves (across cores)
ML = B // GB              # 512 rows per core
JL = J // GJ              # 2048 cols per core
JW = JL // 2              # 1024 cols per wave (local j-half)
P = 128
NB = ML // P              # 4 b-tiles per core
KP = D // 256             # 16 k-pair tiles
JN = 512                  # one PSUM bank
KRET = 4                  # retirement kps (12..15)
WARM = 28                 # PE p-state warm-up matmuls
# mask DMA chunks (in kp units): small first chunks for a fast start,
# 2-kp chunks after to keep the issue count low
MCHUNKS = (1, 1, 2, 2, 2, 2, 2, 2, 2)
# xT DMA chunks (in kp units): small first chunk so the first matmul's
# stationary lands with the first mask chunk
XCHUNKS = (2, 2, 4, 4, 4)

_cache = {}


def _build():
    import concourse.bacc as bacc
    import concourse.mybir as mybir
    import concourse.tile as tile

    dt = mybir.dt
    f8 = dt.float8e4
    f32 = dt.float32
    AF = mybir.ActivationFunctionType
    ALU = mybir.AluOpType
    DR = mybir.MatmulPerfMode.DoubleRow

    nc = bacc.Bacc("TRN2", target_bir_lowering=False, debug=False,
                   num_devices=NCORES)

    xT_ds = [nc.dram_tensor(f"xt{c}", [P, NB, nk, 2, P], f8,
                            kind="ExternalInput")
             for c, nk in enumerate(XCHUNKS)]
    xrm_d = nc.dram_tensor("xrm", [NB, P, D], f8, kind="ExternalInput")
    m_d = nc.dram_tensor("masks", [2, KP, P, 2, JW], dt.uint8,
                         kind="ExternalInput")
    thb_d = nc.dram_tensor("thb", [P, JL], f32, kind="ExternalInput")
    o_d = nc.dram_tensor("out", [NB, P, JL], dt.uint8, kind="ExternalOutput")

    with tile.TileContext(nc) as tc:
        with (
            tc.tile_pool(name="const", bufs=1) as constp,
            tc.tile_pool(name="mask", bufs=1) as maskp,
            tc.tile_pool(name="xt", bufs=1) as xtp,
            tc.tile_pool(name="xrm", bufs=1) as xrmp,
            tc.tile_pool(name="acts", bufs=1) as actp,
            tc.tile_pool(name="tmp", bufs=1) as tmpp,
            tc.tile_pool(name="bound", bufs=1) as boundp,
            tc.tile_pool(name="ob", bufs=1) as obsp,
        ):
            mtc = [[maskp.tile([P, nk, 2, JW], dt.uint8,
                               name=f"mk{hh}_{c}")
                    for c, nk in enumerate(MCHUNKS)] for hh in range(2)]
            kp2c = []
            for c, nk in enumerate(MCHUNKS):
                kp2c += [(c, i) for i in range(nk)]
            xq = [xtp.tile([P, NB, nk, 2, P], f8, name=f"xq{q}")
                  for q, nk in enumerate(XCHUNKS)]
            xbase = []
            k0 = 0
            for nk in XCHUNKS:
                xbase.append(k0)
                k0 += nk
            xrm01 = xrmp.tile([P, 2, D], f8)
            xrm23 = xrmp.tile([P, 2, D], f8)
            thb = constp.tile([P, JL], f32)

            def mte(hh, kp, jj):
                c, i = kp2c[kp]
                return mtc[hh][c][:, i, :, jj:jj + JN]

            def xte(b, kp):
                for q in range(len(XCHUNKS) - 1, -1, -1):
                    if kp >= xbase[q]:
                        return xq[q][:, b, kp - xbase[q]]

            # DMA order: first xT chunk on the scalar queue (lands in
            # parallel with the first mask chunk); the strictly ordered
            # gpsimd queue carries wave-A (j-half 0) masks with the
            # remaining xT chunks slotted before the kp range needing
            # them, then thb/xrm, then wave-B masks.
            nc.scalar.dma_start(xq[0][:], xT_ds[0][:])
            for c, nk in enumerate(MCHUNKS):
                kp0 = kp2c.index((c, 0))
                for q, xb in enumerate(xbase):
                    if q > 0 and xb == kp0:
                        nc.gpsimd.dma_start(xq[q][:], xT_ds[q][:])
                nc.gpsimd.dma_start(
                    mtc[0][c][:], m_d[0, kp0:kp0 + nk].rearrange(
                        "k p a j -> p k a j"))
            nc.gpsimd.dma_start(thb[:], thb_d[:])
            nc.gpsimd.dma_start(
                xrm01[:], xrm_d[0:2].rearrange("b p k -> p b k"))
            nc.gpsimd.dma_start(
                xrm23[:], xrm_d[2:4].rearrange("b p k -> p b k"))
            for c, nk in enumerate(MCHUNKS):
                kp0 = kp2c.index((c, 0))
                nc.gpsimd.dma_start(
                    mtc[1][c][:], m_d[1, kp0:kp0 + nk].rearrange(
                        "k p a j -> p k a j"))

            # ---- constants / warm-up
            wtile = constp.tile([P, 2, P], f8)
            nc.vector.memset(wtile[:], 0.0)
            zero1 = constp.tile([P, 1], f32)
            nc.vector.memset(zero1[:], 0.0)
            neg4 = constp.tile([P, 1], f32)
            nc.vector.memset(neg4[:], -4.0)
            actw = constp.tile([P, 1], f32)
            nc.scalar.activation(actw[:], zero1[:], AF.Identity,
                                 bias=zero1[:], scale=1.0)

            rxa = [constp.tile([P, 1], f32, name=f"rxa{b}")
                   for b in range(NB)]
            rxe = [constp.tile([P, 1], f32, name=f"rxe{b}")
                   for b in range(NB)]
            sc8 = [actp.tile([P, D], f8, name=f"sc8_{i}") for i in range(3)]

            # rowsum reductions: b0/b1 serial on scalar, b2/b3 on vector
            # (emitted later, after the wave-A psum-releasing ops).
            for b in (0, 1):
                nc.scalar.activation(sc8[b][:], xrm01[:, b], AF.Identity,
                                     bias=zero1[:], scale=1.0,
                                     accum_out=rxa[b][:])
                nc.scalar.activation(rxe[b][:], rxa[b][:], AF.Identity,
                                     bias=neg4[:], scale=1.0 / 1024.0)

            obs = [obsp.tile([P, JL], dt.uint8, name=f"ob{b}")
                   for b in range(NB)]

            with tc.tile_pool(name="psacc", bufs=1, space="PSUM") as psacc:
                dps = psacc.tile([P, JN], f32, tag="acc0", name="dps")
                for i in range(WARM):
                    nc.tensor.matmul(dps[:, 0:P], wtile[:], wtile[:],
                                     start=True, stop=True, perf_mode=DR)

                for w in range(2):
                    hh = w
                    ps = {}
                    for b in range(NB):
                        for jl in range(2):
                            ps[(b, jl)] = psacc.tile(
                                [P, JN], f32, tag=f"acc{b * 2 + jl}",
                                name=f"acc_w{w}_{b}_{jl}")
                    # bulk: kp-major over kp 0..11
                    for kp in range(KP - KRET):
                        for b in range(NB):
                            wap = xte(b, kp)
                            for jl in range(2):
                                nc.tensor.matmul(
                                    ps[(b, jl)][:], wap,
                                    mte(hh, kp, jl * JN).bitcast(f8),
                                    start=(kp == 0), stop=False,
                                    perf_mode=DR)
                    # retirement: group-major over kp 12..15, staggered
                    tmps = []
                    for b in range(NB):
                        for jl in range(2):
                            col = hh * JW + jl * JN
                            for kp in range(KP - KRET, KP):
                                nc.tensor.matmul(
                                    ps[(b, jl)][:], xte(b, kp),
                                    mte(hh, kp, jl * JN).bitcast(f8),
                                    start=False, stop=(kp == KP - 1),
                                    perf_mode=DR)
                            if w == 0:
                                # two-op epilogue: op1 (DVE) releases the
                                # psum bank using only thb; op2 (deferred
                                # below) waits on the rowsum path.
                                tmp = tmpp.tile([P, JN], f32,
                                                tag=f"tmp{b * 2 + jl}",
                                                name=f"tmp{b}_{jl}")
                                nc.vector.tensor_tensor(
                                    tmp[:], ps[(b, jl)][:],
                                    thb[:, col:col + JN], op=ALU.subtract)
                                tmps.append((b, col, tmp))
                            else:
                                nc.vector.tensor_tensor(
                                    obs[b][:, col:col + JN], ps[(b, jl)][:],
                                    bound[(b, jl)][:], op=ALU.is_gt)
                                nc.sync.dma_start(o_d[b, :, col:col + JN],
                                                  obs[b][:, col:col + JN])
                    if w == 0:
                        # b2/b3 rowsums, wave-A op2s, wave-B bound tiles
                        # -- all on DVE, emitted after wave-A's op1s so
                        # they can't head-of-line block the psum-bank
                        # releases.
                        for i, b in enumerate((2, 3)):
                            nc.vector.tensor_scalar(
                                sc8[2][:], xrm23[:, i], 1.0, 0.0,
                                op0=ALU.mult, op1=ALU.add,
                                accum_out=rxa[b][:])
                            nc.vector.tensor_scalar(
                                rxe[b][:], rxa[b][:], 1.0 / 1024.0, -4.0,
                                op0=ALU.mult, op1=ALU.add)
                        for b, col, tmp in tmps:
                            nc.vector.tensor_scalar(
                                obs[b][:, col:col + JN], tmp[:],
                                rxe[b][:], None, op0=ALU.is_gt)
                            nc.sync.dma_start(o_d[b, :, col:col + JN],
                                              obs[b][:, col:col + JN])
                        bound = {}
                        for b in range(NB):
                            for jl in range(2):
                                col = JW + jl * JN
                                bt = boundp.tile([P, JN], f32,
                                                 name=f"bnd{b}_{jl}")
                                nc.vector.tensor_scalar(
                                    bt[:], thb[:, col:col + JN],
                                    rxe[b][:], None, op0=ALU.add)
                                bound[(b, jl)] = bt

    nc.compile()
    return nc


def _get_nc():
    if "nc" not in _cache:
        _cache["nc"] = _build()
    return _cache["nc"]


def _prep_core(xs8, mask_buf, thb_buf):
    """Per-core input dict from the fp8 x slice and shared mask/th bufs."""
    out = {
        "xrm": np.ascontiguousarray(xs8.reshape(NB, P, D)),
        "masks": mask_buf,
        "thb": thb_buf,
    }
    xb = 0
    for c, nk in enumerate(XCHUNKS):
        sl = xs8[:, xb * 256:(xb + nk) * 256].reshape(NB, P, nk, 2, P)
        out[f"xt{c}"] = np.ascontiguousarray(
            sl.transpose(4, 0, 2, 3, 1))         # [ki, b, kc, ko, m]
        xb += nk
    return out


def run(x, masks, thresholds, trace=False):
    """Run the SPMD kernel on 8 cores. Returns (out_bool, results)."""
    import ml_dtypes
    from concourse.bass_utils import run_bass_kernel_spmd

    nc = _get_nc()
    f8 = ml_dtypes.float8_e4m3

    xs8_all = np.where(np.asarray(x) != 0, np.float32(1.0),
                       np.float32(-1.0)).astype(f8)
    m_u8 = np.ascontiguousarray(np.asarray(masks).view(np.uint8))
    th = np.asarray(thresholds).astype(np.float32) * np.float32(2.0 ** -9)

    mask_bufs, thb_bufs = [], []
    for h in range(GJ):
        mh = m_u8[:, h * JL:(h + 1) * JL].reshape(KP, 2, P, 2, JW)
        mask_bufs.append(np.ascontiguousarray(
            mh.transpose(3, 0, 2, 1, 4)))        # [hh, kp, ki, ko, j]
        thb_bufs.append(np.ascontiguousarray(
            np.broadcast_to(th[None, h * JL:(h + 1) * JL], (P, JL))))

    in_maps = []
    for c in range(NCORES):
        g, h = c // GJ, c % GJ
        in_maps.append(_prep_core(xs8_all[g * ML:(g + 1) * ML],
                                  mask_bufs[h], thb_bufs[h]))

    res = run_bass_kernel_spmd(nc, in_maps, core_ids=list(range(NCORES)),
                               trace=trace)
    out = np.empty((B, J), dtype=np.uint8)
    for c in range(NCORES):
        g, h = c // GJ, c % GJ
        out[g * ML:(g + 1) * ML, h * JL:(h + 1) * JL] = \
            res.results[c]["out"].reshape(ML, JL)
    return out.view(np.bool_), res


def kernel(x, masks, thresholds):
    x = np.asarray(x)
    masks = np.asarray(masks)
    thresholds = np.asarray(thresholds)
    out, _ = run(x, masks, thresholds, trace=False)
    return out
